# revision 31
# baseline (speedup 1.0000x reference)
"""Trainium2 Bass kernel for deformable attention.

Contract: kernel(**inputs) takes the FULL inputs (as produced by the problem's
setup_inputs) and returns the FULL [4, 1024, 256] float32 output. Internally the
work is sharded over 8 NeuronCores: core c handles batch c//2 and query half
c%2 (512 queries), with the batch's full value feature map replicated on the
core.

Per-core pipeline (all shapes hardcoded for B=4, Q=1024, D=256, H=W=128,
nh=8, npts=4):
  1. The value projection W_v commutes past the (linear) bilinear/attention
     reduce, so it is folded into the output projection on the host:
     Wcomb_h = W_v @ W_out_h and bvW_h = b_v @ W_out_h, with a per-(q,h)
     sum-of-weights term correcting the bias at zero-padded borders. The
     kernel therefore gathers raw bf16 value rows -- no feature-map GEMM.
  2. Per q-tile of 128 queries: coefficient GEMMs + softmax + bilinear
     weights in [query-partition, sample-free] layout, fp32. The gather
     index path is latency-critical, so indices are built via a replicated
     PE transpose (8x-tiled columns) straight into the SWDGE-wrapped
     layout -- no strided 2-byte-descriptor DMAs.
  3. Gather: per (query, head, point, row-corner) descriptor, one dma_gather
     element of 512 bf16 values = two adjacent columns at one row of the
     value map (1024 idxs per call -- larger calls crash the hardware).
     Chunks of a q-tile issue immediately after that tile's index path, so
     the DMA engines stream gathers from ~6us onward.
  4. Weighted reduce on the TensorEngine: the 128 gathered slots of a query
     pair are the contraction dim; the moving operand is a masked
     block-diagonal [128, 16] weight matrix from bilinear*attention weights.
     Output lands as [d, (q, h)] in PSUM, the lhsT layout the final GEMM
     needs. PSUM->SBUF eviction runs on the Activation engine.
  5. Per q-tile: out = weighted @ Wcomb + sw * bvW + b_out, overlapped with
     the next tile's gathers.
"""

from contextlib import ExitStack

import numpy as np
import ml_dtypes

NH, NPTS = 8, 4
D = 256
HW = 128            # H == W == 128
NROWS = HW * HW     # 16384
QPC = 512           # queries per core
NCORES = 8
NPAIRS = QPC // 2   # 256 query pairs
NCHUNK = 32         # gather chunks (>1024 idxs per dma_gather crashes HW)
PAIRS_PER_CHUNK = NPAIRS // NCHUNK  # 8
IDX_PER_CHUNK = PAIRS_PER_CHUNK * 128  # 1024

_CACHE = {}


def _mask16_np():
    """[128, 16] bf16: mask[qq*64 + h*8 + p*2 + yp, qq*8 + h] = 1."""
    m = np.zeros((128, 16), dtype=np.float32)
    for qq in range(2):
        for h in range(NH):
            for p in range(NPTS):
                for yp in range(2):
                    m[qq * 64 + h * 8 + p * 2 + yp, qq * 8 + h] = 1.0
    return m.astype(ml_dtypes.bfloat16)


def _build_bass():
    import concourse.bass as bass
    import concourse.bacc as bacc
    import concourse.mybir as mybir
    import concourse.tile as tile
    from concourse.masks import make_identity

    f32 = mybir.dt.float32
    bf16 = mybir.dt.bfloat16
    i16 = mybir.dt.int16
    i32 = mybir.dt.int32
    Alu = mybir.AluOpType
    Act = mybir.ActivationFunctionType

    nc = bacc.Bacc("TRN2", target_bir_lowering=False,
                   dynamic_dma_scratch_size=32768)

    # ---- I/O ----
    # query arrives host-transposed as [2, 128, QPC]: qT[t, p, q] = q[q, 128t+p]
    # weights arrive host-packed (see _make_in_maps) to minimize DMA count:
    #   Wcat  [128, 2, 96] f32: W_off*12.8 | W_attn, dim-major
    #   biasp [1, 352]     f32: b_off*12.8 | b_attn | b_out
    #   Wout  [128, 17, 256] bf16: Wcomb planes 0-15, bvW plane 16 (rows 0-7)
    #   maskd [128, 16]      bf16: block-diagonal reduce mask
    query = nc.dram_tensor("query", [2, 128, QPC], f32, kind="ExternalInput")
    refp = nc.dram_tensor("reference_points", [QPC, 2], f32, kind="ExternalInput")
    value = nc.dram_tensor("value", [NROWS, D], bf16, kind="ExternalInput")
    Wcat = nc.dram_tensor("Wcat", [128, 2, 96], f32, kind="ExternalInput")
    biasp = nc.dram_tensor("biasp", [1, 352], f32, kind="ExternalInput")
    Wout = nc.dram_tensor("Wout", [128, 17, 256], bf16, kind="ExternalInput")
    maskd = nc.dram_tensor("maskd", [128, 16], bf16, kind="ExternalInput")
    out = nc.dram_tensor("out", [QPC, D], f32, kind="ExternalOutput")

    with tile.TileContext(nc) as tc, ExitStack() as ctx:
        sb = ctx.enter_context(tc.tile_pool(name="sb", bufs=1))
        ps = ctx.enter_context(tc.tile_pool(name="ps", bufs=1, space="PSUM"))

        # ---- constants / small weights; queues chosen so SP starts with the
        # qt0 query load and the Act queue stays clear for the idx path ----
        # qt0's query load goes out first on the SP/HWDGE queue -- it heads
        # the gather-index critical path
        q_sbs, rps = [], []
        for qt in range(4):
            qT = sb.tile([128, 2, 128], f32, tag="qT", bufs=4)
            rp = sb.tile([128, 2], f32, tag="rp", bufs=4)
            q_sbs.append(qT)
            rps.append(rp)
        wcat = sb.tile([128, 2, 96], f32, tag="wcat")
        bias_cat = sb.tile([1, 352], f32, tag="bias_cat")
        nc.sync.dma_start(q_sbs[0][:], query[:, :, 0:128].rearrange("t p q -> p t q"))
        nc.sync.dma_start(wcat[:], Wcat[:])
        nc.sync.dma_start(rps[0][:], refp[0:128, :])
        mask16 = sb.tile([128, 16], bf16, tag="mask16")
        nc.sync.dma_start(mask16[:], maskd[:])
        nc.scalar.dma_start(bias_cat[:], biasp[:])
        # packed Wcomb|bvW load: WAW-pin to the wcat transfer so its 3.2us
        # fills the otherwise-idle DMA window during the coefficient phase
        # without ever delaying the wcat load itself
        wout_bf = sb.tile([128, 17, 256], bf16, tag="wout")
        nc.vector.tensor_copy(wout_bf[0:1, 0, 0:1], wcat[0:1, 0, 0:1])
        nc.scalar.dma_start(wout_bf[:], Wout[:])

        ident = sb.tile([128, 128], f32, tag="ident")
        make_identity(nc, ident[:])
        ones1 = sb.tile([1, 128], f32, tag="ones1")
        nc.vector.memset(ones1[:], 1.0)


        # persistent intermediates
        wabT = sb.tile([128, 512], f32, tag="wabT")       # [(AB,h,p,yp), q]
        w_a_i = sb.tile([128, 256], bf16, tag="w_a_i")    # [(qq,s64), pair]
        w_b_i = sb.tile([128, 256], bf16, tag="w_b_i")
        idxt = sb.tile([128, 256, 2, 4], i16, tag="idxt")  # wrapped gather indices
        red = sb.tile([128, 2, 512, 8], bf16, tag="red")  # [dlo, dh, q, h]
        swT = sb.tile([8, 512], bf16, tag="swT")          # sum of weights [h, q]

        import concourse.bass as bass_mod
        gather_src = bass_mod.AP(
            tensor=value, offset=0, ap=[[256, NROWS - 1], [1, 512]])

        # ============== per q-tile pipeline (4 tiles of 128 queries) ========
        for qt in range(4):
            qsl = slice(qt * 128, (qt + 1) * 128)
            jsl = slice(qt * 64, (qt + 1) * 64)

            # ---- pass 1: coefficient GEMM + gather-index path ----
            qT, rp = q_sbs[qt], rps[qt]
            if qt > 0:
                nc.sync.dma_start(qT[:], query[:, :, qsl].rearrange("t p q -> p t q"))
                nc.sync.dma_start(rp[:], refp[qsl, :])
            rp128 = sb.tile([128, 2], f32, tag="rp128", bufs=4)
            nc.vector.tensor_scalar(rp128[:], rp[:], 128.0, 127.5, Alu.mult, Alu.add)

            if qt == 0:
                # keep the PE busy from t~2us so the coefficient GEMM and the
                # index transposes run at a warm p-state, not the cold 0.65GHz
                dmy = ps.tile([128, 128], f32, tag="tp", bufs=2)
                for _ in range(14):
                    nc.tensor.transpose(dmy[:, 0:128], ident[:], ident[:])

            psc = ps.tile([128, 96], f32, tag="tp", bufs=2)
            nc.tensor.matmul(psc[:], qT[:, 0, :], wcat[:, 0, :], start=True, stop=False)
            nc.tensor.matmul(psc[:], qT[:, 1, :], wcat[:, 1, :], start=False, stop=False)
            nc.tensor.matmul(psc[:], ones1[:], bias_cat[:, 0:96], start=False, stop=True)
            coef = sb.tile([128, 96], f32, tag="coef_sb", bufs=4)
            nc.vector.tensor_copy(coef[:], psc[:])

            # sampling grid -> pixel coords + 128, x/y interleaved
            pxs = sb.tile([128, 64], f32, tag="pxs", bufs=4)  # px + 128
            nc.vector.tensor_tensor(
                pxs[:].rearrange("p (s c) -> p s c", c=2),
                coef[:, 0:64].rearrange("p (s c) -> p s c", c=2),
                rp128[:, None, :].to_broadcast([128, 32, 2]), Alu.add)
            nc.vector.tensor_scalar(pxs[:], pxs[:], 127.5, 255.5, Alu.max, Alu.min)
            # pxs >= 127.5 > 0, so the f32->i32 truncation IS the floor
            ri = sb.tile([128, 64], i32, tag="ri", bufs=4)
            nc.vector.tensor_copy(ri[:], pxs[:])
            flr = sb.tile([128, 64], f32, tag="flr", bufs=4)  # floor(px) + 128
            nc.vector.tensor_copy(flr[:], ri[:])
            st = sb.tile([128, 64], f32, tag="st", bufs=4)    # clip start + 128
            nc.vector.tensor_scalar(st[:], flr[:], 128.0, 254.0, Alu.max, Alu.min)

            # gather indices: idx64[q, s, A/B-row] then transpose+interleave
            tbase = sb.tile([128, 32], f32, tag="tbase", bufs=4)
            nc.vector.tensor_scalar(
                tbase[:], st[:].rearrange("p (s c) -> p s c", c=2)[:, :, 1],
                128.0, -16512.0, Alu.mult, Alu.add)
            idx64 = sb.tile([128, 32, 2], f32, tag="idx64", bufs=4)
            nc.vector.tensor_tensor(idx64[:, :, 0], tbase[:],
                                    st[:].rearrange("p (s c) -> p s c", c=2)[:, :, 0], Alu.add)
            nc.vector.tensor_scalar_add(idx64[:, :, 1], idx64[:, :, 0], 128.0)

            # transpose each 16-column idx group with its columns 8x-tiled
            # (stride-0 broadcast), landing the group replicated across all
            # 128 partitions -- the SWDGE wrapped-and-replicated idx layout
            pti_a = ps.tile([128, 2, 128], f32, tag="tp", bufs=2)
            pti_b = ps.tile([128, 2, 128], f32, tag="tp", bufs=2)
            idx64f = idx64[:].rearrange("p s c -> p (s c)")
            for g4 in range(4):
                pti = pti_a if g4 < 2 else pti_b
                nc.tensor.transpose(
                    pti[:, g4 % 2, :],
                    idx64f[:, None, 16 * g4:16 * (g4 + 1)].to_broadcast([128, 8, 16]),
                    ident[:])
            nc.vector.tensor_copy(
                idxt[:, jsl, :, 0:2],
                pti_a[:].rearrange("p g (j q) -> p j q g", q=2))
            nc.vector.tensor_copy(
                idxt[:, jsl, :, 2:4],
                pti_b[:].rearrange("p g (j q) -> p j q g", q=2))



            # ---- pass 2: softmax + bilinear weights ----
            expw = sb.tile([128, 8, 4], f32, tag="expw", bufs=4)
            nc.scalar.activation(expw[:], coef[:, 64:96], Act.Exp)
            den = sb.tile([128, 8], f32, tag="den", bufs=4)
            nc.vector.tensor_reduce(den[:], expw[:], axis=mybir.AxisListType.X, op=Alu.add)
            rden = sb.tile([128, 8], f32, tag="rden", bufs=4)
            nc.vector.reciprocal(rden[:], den[:])
            attn = sb.tile([128, 32], f32, tag="attn", bufs=4)
            nc.vector.tensor_tensor(
                attn[:].rearrange("p (h f) -> p h f", f=4), expw[:],
                rden[:, :, None].to_broadcast([128, 8, 4]), Alu.mult)
            w1 = sb.tile([128, 64], f32, tag="w1", bufs=4)
            nc.vector.tensor_tensor(w1[:], pxs[:], flr[:], Alu.subtract)
            dd = sb.tile([128, 64], f32, tag="dd", bufs=4)
            nc.vector.tensor_tensor(dd[:], flr[:], st[:], Alu.subtract)
            m0 = sb.tile([128, 64], f32, tag="m0", bufs=4)
            nc.vector.tensor_scalar(m0[:], dd[:], 0.0, None, Alu.is_equal)
            mneg = sb.tile([128, 64], f32, tag="mneg", bufs=4)
            nc.vector.tensor_scalar(mneg[:], dd[:], -1.0, None, Alu.is_equal)
            mpos = sb.tile([128, 64], f32, tag="mpos", bufs=4)
            nc.vector.tensor_scalar(mpos[:], dd[:], 1.0, None, Alu.is_equal)
            u0 = sb.tile([128, 64], f32, tag="u0", bufs=4)
            nc.vector.tensor_scalar(u0[:], w1[:], 1.0, -1.0, Alu.subtract, Alu.mult)
            # wA = u0*m0 + u1*mneg ; wB = u1*m0 + u0*mpos   (u1 == w1)
            tA = sb.tile([128, 64], f32, tag="tA", bufs=4)
            nc.vector.tensor_tensor(tA[:], u0[:], m0[:], Alu.mult)
            tB = sb.tile([128, 64], f32, tag="tB", bufs=4)
            nc.vector.tensor_tensor(tB[:], w1[:], mneg[:], Alu.mult)
            wA = sb.tile([128, 32, 2], f32, tag="wA", bufs=4)
            nc.vector.tensor_tensor(wA[:].rearrange("p a b -> p (a b)"), tA[:], tB[:], Alu.add)
            nc.vector.tensor_tensor(tA[:], w1[:], m0[:], Alu.mult)
            nc.vector.tensor_tensor(tB[:], u0[:], mpos[:], Alu.mult)
            wB = sb.tile([128, 32, 2], f32, tag="wB", bufs=4)
            nc.vector.tensor_tensor(wB[:].rearrange("p a b -> p (a b)"), tA[:], tB[:], Alu.add)

            # combine with attention; build wab [128, (AB, h*p, yp)]
            aw = sb.tile([128, 32], f32, tag="aw", bufs=4)
            nc.vector.tensor_tensor(aw[:], attn[:], wA[:, :, 0], Alu.mult)
            bw = sb.tile([128, 32], f32, tag="bw", bufs=4)
            nc.vector.tensor_tensor(bw[:], attn[:], wB[:, :, 0], Alu.mult)
            vcat = sb.tile([128, 32, 2], f32, tag="vcat", bufs=4)
            nc.vector.tensor_copy(vcat[:, :, 0], wA[:, :, 1])
            nc.vector.tensor_copy(vcat[:, :, 1], wB[:, :, 1])
            wab = sb.tile([128, 2, 32, 2], f32, tag="wab", bufs=4)
            nc.vector.tensor_tensor(wab[:, 0], vcat[:],
                                    aw[:, :, None].to_broadcast([128, 32, 2]), Alu.mult)
            nc.vector.tensor_tensor(wab[:, 1], vcat[:],
                                    bw[:, :, None].to_broadcast([128, 32, 2]), Alu.mult)

            # sum of all weights per (q, h) -- border-clip correction for the
            # folded b_v term: sw = sum_{AB,p,yp} wab
            swq = sb.tile([128, 8], f32, tag="swq", bufs=4)
            nc.vector.tensor_reduce(
                swq[:], wab[:].rearrange("p a (h r) c -> p h a r c", h=8),
                axis=mybir.AxisListType.XYZ, op=Alu.add)

            pst2 = ps.tile([128, 256], f32, tag="tp", bufs=2)
            nc.tensor.transpose(pst2[:, 0:128],
                                wab[:].rearrange("p a s c -> p (a s c)"), ident[:])
            pst3 = ps.tile([8, 128], f32, tag="img", bufs=2)
            nc.tensor.transpose(pst3[:], swq[:], ident[:])
            nc.vector.tensor_copy(wabT[:, qsl], pst2[:, 0:128])
            nc.vector.tensor_copy(swT[:, qsl], pst3[:])
            nc.vector.tensor_copy(w_a_i[0:64, jsl], wabT[0:64, qt * 128:qt * 128 + 128:2])
            nc.vector.tensor_copy(w_a_i[64:128, jsl], wabT[0:64, qt * 128 + 1:qt * 128 + 128:2])
            nc.vector.tensor_copy(w_b_i[0:64, jsl], wabT[64:128, qt * 128:qt * 128 + 128:2])
            nc.vector.tensor_copy(w_b_i[64:128, jsl], wabT[64:128, qt * 128 + 1:qt * 128 + 128:2])

            # ---- gather + weighted reduce for this q-tile's 8 chunks;
            # the output GEMM runs per half-tile (64 queries): per-chunk-pair
            # splits lose to the per-kt Ldweights cost on the PE ----
            def out_gemm_half(half):
                hsl = slice(qt * 128 + 64 * half, qt * 128 + 64 * (half + 1))
                pso = ps.tile([64, 256], f32, tag="img", bufs=2)
                for kt, (dh, h) in enumerate((dh, h) for dh in range(2)
                                             for h in range(NH)):
                    nc.tensor.matmul(pso[:], red[:, dh, hsl, h],
                                     wout_bf[:, 2 * h + dh, :],
                                     start=(kt == 0), stop=False)
                nc.tensor.matmul(pso[:], swT[:, hsl], wout_bf[0:8, 16, :],
                                 start=False, stop=False)
                nc.tensor.matmul(pso[:], ones1[:, 0:64], bias_cat[:, 96:352],
                                 start=False, stop=True)
                o_sb = sb.tile([64, 256], f32, tag="o_sb", bufs=2)
                nc.vector.tensor_copy(o_sb[:], pso[:])
                nc.sync.dma_start(out[hsl, :], o_sb[:])

            for g in range(8 * qt, 8 * qt + 8):
                gsl = slice(g * PAIRS_PER_CHUNK, (g + 1) * PAIRS_PER_CHUNK)
                gt_sb = sb.tile([128, PAIRS_PER_CHUNK, 512], bf16, tag="gat", bufs=12)
                if g == NCHUNK - 1:
                    # warm the PE during the last gather so the final reduce
                    # and output GEMM run at full clock; reading the previous
                    # chunk's gathered tile pins these in-phase (dep-free
                    # dummies get hoisted early by the scheduler)
                    dmy2 = ps.tile([128, 256], f32, tag="tp", bufs=2)
                    for r in range(18):
                        nc.tensor.matmul(dmy2[:], prev_gt[:, r % 8, 0:128],
                                         wout_bf[:, r % 16, :],
                                         start=True, stop=True)
                nc.gpsimd.dma_gather(
                    out_ap=gt_sb[:],
                    in_ap=gather_src,
                    idxs_ap=idxt[:].rearrange("p a b c -> p (a b c)")[
                        :, g * (IDX_PER_CHUNK // 16):(g + 1) * (IDX_PER_CHUNK // 16)],
                    num_idxs=IDX_PER_CHUNK,
                    num_idxs_reg=IDX_PER_CHUNK,
                    elem_size=512,
                    elem_step=256,
                )
                wblkA = sb.tile([128, PAIRS_PER_CHUNK, 16], bf16, tag="wblkA", bufs=4)
                nc.vector.tensor_tensor(
                    wblkA[:], mask16[:, None, :].to_broadcast([128, PAIRS_PER_CHUNK, 16]),
                    w_a_i[:, gsl, None].to_broadcast([128, PAIRS_PER_CHUNK, 16]), Alu.mult)
                wblkB = sb.tile([128, PAIRS_PER_CHUNK, 16], bf16, tag="wblkB", bufs=4)
                nc.vector.tensor_tensor(
                    wblkB[:], mask16[:, None, :].to_broadcast([128, PAIRS_PER_CHUNK, 16]),
                    w_b_i[:, gsl, None].to_broadcast([128, PAIRS_PER_CHUNK, 16]), Alu.mult)

                plo = ps.tile([128, PAIRS_PER_CHUNK * 16], f32, tag="red_lo", bufs=2)
                phi = ps.tile([128, PAIRS_PER_CHUNK * 16], f32, tag="red_hi", bufs=2)
                qsl2 = slice(g * 2 * PAIRS_PER_CHUNK, (g + 1) * 2 * PAIRS_PER_CHUNK)
                for j in range(PAIRS_PER_CHUNK):
                    osl = slice(j * 16, (j + 1) * 16)
                    nc.tensor.matmul(plo[:, osl], gt_sb[:, j, 0:128], wblkA[:, j, :],
                                     start=True, stop=False)
                    nc.tensor.matmul(plo[:, osl], gt_sb[:, j, 256:384], wblkB[:, j, :],
                                     start=False, stop=True)
                nc.scalar.copy(
                    red[:, 0, qsl2, :].rearrange("p a b -> p (a b)"), plo[:])
                for j in range(PAIRS_PER_CHUNK):
                    osl = slice(j * 16, (j + 1) * 16)
                    nc.tensor.matmul(phi[:, osl], gt_sb[:, j, 128:256], wblkA[:, j, :],
                                     start=True, stop=False)
                    nc.tensor.matmul(phi[:, osl], gt_sb[:, j, 384:512], wblkB[:, j, :],
                                     start=False, stop=True)
                nc.scalar.copy(
                    red[:, 1, qsl2, :].rearrange("p a b -> p (a b)"), phi[:])
                if g % 8 == 3:
                    out_gemm_half(0)
                elif g % 8 == 7:
                    out_gemm_half(1)
                prev_gt = gt_sb

    nc.compile()
    return nc


def _get_nc():
    if "nc" not in _CACHE:
        _CACHE["nc"] = _build_bass()
    return _CACHE["nc"]


def _make_in_maps(inputs):
    query = np.ascontiguousarray(np.asarray(inputs["query"], dtype=np.float32))
    refp = np.ascontiguousarray(np.asarray(inputs["reference_points"], dtype=np.float32))
    value = np.ascontiguousarray(
        np.asarray(inputs["value"], dtype=np.float32).astype(ml_dtypes.bfloat16))

    # Wcat [128, 2, 96]: dim-major [p, t, n] view of [W_off*12.8 | W_attn]
    W_off = np.asarray(inputs["W_off"], np.float32) * 12.8
    W_attn = np.asarray(inputs["W_attn"], np.float32)
    wcat = np.concatenate([W_off, W_attn], axis=1)  # [256, 96]
    wcat = wcat.reshape(2, 128, 96).transpose(1, 0, 2)  # [p, t, n]

    # biasp [1, 352]: b_off*12.8 | b_attn | b_out
    biasp = np.concatenate([
        np.asarray(inputs["b_off"], np.float32) * 12.8,
        np.asarray(inputs["b_attn"], np.float32),
        np.asarray(inputs["b_out"], np.float32)])[None, :]

    # Wout [128, 18, 256] bf16: Wcomb planes 0-15 ([p, t, n] of [2048, 256]),
    # bvW plane 16 (rows 0-7), reduce mask plane 17 (cols 0-15)
    W_v = np.asarray(inputs["W_v"], np.float64)
    b_v = np.asarray(inputs["b_v"], np.float64)
    W_out = np.asarray(inputs["W_out"], np.float64).reshape(NH, D, D)
    Wcomb = np.einsum("ij,hjk->hik", W_v, W_out).reshape(NH * D, D)
    bvW = np.einsum("j,hjk->hk", b_v, W_out)
    wout = np.zeros((128, 17, 256), np.float32)
    wout[:, 0:16, :] = Wcomb.reshape(16, 128, 256).transpose(1, 0, 2)
    wout[0:8, 16, :] = bvW

    consts = {
        "Wcat": np.ascontiguousarray(wcat),
        "biasp": np.ascontiguousarray(biasp),
        "Wout": np.ascontiguousarray(wout.astype(ml_dtypes.bfloat16)),
        "maskd": np.ascontiguousarray(_mask16_np()),
    }
    in_maps = []
    for c in range(NCORES):
        b, s = c // 2, c % 2
        qsl = slice(s * QPC, (s + 1) * QPC)
        in_maps.append({
            # host-transposed layout [2, 128, QPC]: qT[t, p, q] = q[q, 128t+p]
            "query": np.ascontiguousarray(
                query[b, qsl].reshape(QPC, 2, 128).transpose(1, 2, 0)),
            "reference_points": np.ascontiguousarray(refp[b, qsl]),
            "value": np.ascontiguousarray(value[b]),
            **consts,
        })
    return in_maps


def _assemble(outs, shape):
    out = np.zeros(shape, dtype=np.float32)
    for c in range(NCORES):
        b, s = c // 2, c % 2
        out[b, s * QPC:(s + 1) * QPC] = outs[c]["out"]
    return out


def kernel(query, reference_points, value, W_off, b_off, W_attn, b_attn,
           W_v, b_v, W_out, b_out, H=128, W=128, **_unused):
    assert int(H) == HW and int(W) == HW
    from concourse.bass_utils import run_bass_kernel_spmd

    inputs = dict(query=query, reference_points=reference_points, value=value,
                  W_off=W_off, b_off=b_off, W_attn=W_attn, b_attn=b_attn,
                  W_v=W_v, b_v=b_v, W_out=W_out, b_out=b_out)
    in_maps = _make_in_maps(inputs)
    nc = _get_nc()
    res = run_bass_kernel_spmd(nc, in_maps, core_ids=list(range(NCORES)))
    outs = res.results if hasattr(res, "results") else res
    B, Q, _ = np.asarray(query).shape
    return _assemble(outs, (B, Q, D))


# revision 32
# speedup vs baseline: 1.0158x; 1.0158x over previous
"""Trainium2 Bass kernel for deformable attention.

Contract: kernel(**inputs) takes the FULL inputs (as produced by the problem's
setup_inputs) and returns the FULL [4, 1024, 256] float32 output. Internally the
work is sharded over 8 NeuronCores: core c handles batch c//2 and query half
c%2 (512 queries), with the batch's full value feature map replicated on the
core.

Per-core pipeline (all shapes hardcoded for B=4, Q=1024, D=256, H=W=128,
nh=8, npts=4):
  1. The value projection W_v commutes past the (linear) bilinear/attention
     reduce, so it is folded into the output projection on the host:
     Wcomb_h = W_v @ W_out_h and bvW_h = b_v @ W_out_h, with a per-(q,h)
     sum-of-weights term correcting the bias at zero-padded borders. The
     kernel therefore gathers raw bf16 value rows -- no feature-map GEMM.
  2. Per q-tile of 128 queries: coefficient GEMMs + softmax + bilinear
     weights in [query-partition, sample-free] layout, fp32. The gather
     index path is latency-critical, so indices are built via a replicated
     PE transpose (8x-tiled columns) straight into the SWDGE-wrapped
     layout -- no strided 2-byte-descriptor DMAs.
  3. Gather: per (query, head, point, row-corner) descriptor, one dma_gather
     element of 512 bf16 values = two adjacent columns at one row of the
     value map (1024 idxs per call -- larger calls crash the hardware).
     Chunks of a q-tile issue immediately after that tile's index path, so
     the DMA engines stream gathers from ~6us onward.
  4. Weighted reduce on the TensorEngine: the 128 gathered slots of a query
     pair are the contraction dim; the moving operand is a masked
     block-diagonal [128, 16] weight matrix from bilinear*attention weights.
     Output lands as [d, (q, h)] in PSUM, the lhsT layout the final GEMM
     needs. PSUM->SBUF eviction runs on the Activation engine.
  5. Per q-tile: out = weighted @ Wcomb + sw * bvW + b_out, overlapped with
     the next tile's gathers.
"""

from contextlib import ExitStack

import numpy as np
import ml_dtypes

NH, NPTS = 8, 4
D = 256
HW = 128            # H == W == 128
NROWS = HW * HW     # 16384
QPC = 512           # queries per core
NCORES = 8
NPAIRS = QPC // 2   # 256 query pairs
NCHUNK = 32         # gather chunks (>1024 idxs per dma_gather crashes HW)
PAIRS_PER_CHUNK = NPAIRS // NCHUNK  # 8
IDX_PER_CHUNK = PAIRS_PER_CHUNK * 128  # 1024

_CACHE = {}


def _mask16_np():
    """[128, 16] bf16: mask[qq*64 + h*8 + p*2 + yp, qq*8 + h] = 1."""
    m = np.zeros((128, 16), dtype=np.float32)
    for qq in range(2):
        for h in range(NH):
            for p in range(NPTS):
                for yp in range(2):
                    m[qq * 64 + h * 8 + p * 2 + yp, qq * 8 + h] = 1.0
    return m.astype(ml_dtypes.bfloat16)


def _build_bass():
    import concourse.bass as bass
    import concourse.bacc as bacc
    import concourse.mybir as mybir
    import concourse.tile as tile
    from concourse.masks import make_identity

    f32 = mybir.dt.float32
    bf16 = mybir.dt.bfloat16
    i16 = mybir.dt.int16
    i32 = mybir.dt.int32
    Alu = mybir.AluOpType
    Act = mybir.ActivationFunctionType

    nc = bacc.Bacc("TRN2", target_bir_lowering=False,
                   dynamic_dma_scratch_size=32768)

    # ---- I/O ----
    # query arrives host-transposed as [2, 128, QPC]: qT[t, p, q] = q[q, 128t+p]
    # weights arrive host-packed (see _make_in_maps) to minimize DMA count:
    #   Wcat  [128, 2, 96] f32: W_off*12.8 | W_attn, dim-major
    #   biasp [1, 352]     f32: b_off*12.8 | b_attn | b_out
    #   Wout  [128, 17, 256] bf16: Wcomb planes 0-15, bvW plane 16 (rows 0-7)
    #   maskd [128, 16]      bf16: block-diagonal reduce mask
    query = nc.dram_tensor("query", [2, 128, QPC], f32, kind="ExternalInput")
    refp = nc.dram_tensor("reference_points", [QPC, 2], f32, kind="ExternalInput")
    value = nc.dram_tensor("value", [NROWS, D], bf16, kind="ExternalInput")
    Wcat = nc.dram_tensor("Wcat", [128, 2, 96], f32, kind="ExternalInput")
    biasp = nc.dram_tensor("biasp", [1, 352], f32, kind="ExternalInput")
    Wout = nc.dram_tensor("Wout", [128, 17, 256], bf16, kind="ExternalInput")
    maskd = nc.dram_tensor("maskd", [128, 16], bf16, kind="ExternalInput")
    out = nc.dram_tensor("out", [QPC, D], f32, kind="ExternalOutput")

    with tile.TileContext(nc) as tc, ExitStack() as ctx:
        sb = ctx.enter_context(tc.tile_pool(name="sb", bufs=1))
        ps = ctx.enter_context(tc.tile_pool(name="ps", bufs=1, space="PSUM"))

        # ---- constants / small weights; queues chosen so SP starts with the
        # qt0 query load and the Act queue stays clear for the idx path ----
        # qt0's query load goes out first on the SP/HWDGE queue -- it heads
        # the gather-index critical path
        q_sbs, rps = [], []
        for qt in range(4):
            qT = sb.tile([128, 2, 128], f32, tag="qT", bufs=4)
            rp = sb.tile([128, 2], f32, tag="rp", bufs=4)
            q_sbs.append(qT)
            rps.append(rp)
        wcat = sb.tile([128, 2, 96], f32, tag="wcat")
        bias_cat = sb.tile([1, 352], f32, tag="bias_cat")
        nc.sync.dma_start(q_sbs[0][:], query[:, :, 0:128].rearrange("t p q -> p t q"))
        nc.sync.dma_start(wcat[:], Wcat[:])
        nc.sync.dma_start(rps[0][:], refp[0:128, :])
        mask16 = sb.tile([128, 16], bf16, tag="mask16")
        nc.sync.dma_start(mask16[:], maskd[:])
        nc.scalar.dma_start(bias_cat[:], biasp[:])
        wout_bf = sb.tile([128, 17, 256], bf16, tag="wout")

        ident = sb.tile([128, 128], f32, tag="ident")
        make_identity(nc, ident[:])
        ones1 = sb.tile([1, 128], f32, tag="ones1")
        nc.vector.memset(ones1[:], 1.0)


        # persistent intermediates
        wabT = sb.tile([128, 512], f32, tag="wabT")       # [(AB,h,p,yp), q]
        w_a_i = sb.tile([128, 256], bf16, tag="w_a_i")    # [(qq,s64), pair]
        w_b_i = sb.tile([128, 256], bf16, tag="w_b_i")
        idxt = sb.tile([128, 256, 2, 4], i16, tag="idxt")  # wrapped gather indices
        red = sb.tile([128, 2, 512, 8], bf16, tag="red")  # [dlo, dh, q, h]
        swT = sb.tile([8, 512], bf16, tag="swT")          # sum of weights [h, q]

        import concourse.bass as bass_mod
        gather_src = bass_mod.AP(
            tensor=value, offset=0, ap=[[256, NROWS - 1], [1, 512]])

        # ============== per q-tile pipeline (4 tiles of 128 queries) ========
        for qt in range(4):
            qsl = slice(qt * 128, (qt + 1) * 128)
            jsl = slice(qt * 64, (qt + 1) * 64)

            # ---- pass 1: coefficient GEMM + gather-index path ----
            qT, rp = q_sbs[qt], rps[qt]
            if qt > 0:
                nc.sync.dma_start(qT[:], query[:, :, qsl].rearrange("t p q -> p t q"))
                nc.sync.dma_start(rp[:], refp[qsl, :])
            rp128 = sb.tile([128, 2], f32, tag="rp128", bufs=4)
            nc.vector.tensor_scalar(rp128[:], rp[:], 128.0, 127.5, Alu.mult, Alu.add)

            if qt == 0:
                # keep the PE busy from t~2us so the coefficient GEMM and the
                # index transposes run at a warm p-state, not the cold 0.65GHz
                dmy = ps.tile([128, 128], f32, tag="tp", bufs=2)
                for _ in range(14):
                    nc.tensor.transpose(dmy[:, 0:128], ident[:], ident[:])

            psc = ps.tile([128, 96], f32, tag="tp", bufs=2)
            nc.tensor.matmul(psc[:], qT[:, 0, :], wcat[:, 0, :], start=True, stop=False)
            nc.tensor.matmul(psc[:], qT[:, 1, :], wcat[:, 1, :], start=False, stop=False)
            nc.tensor.matmul(psc[:], ones1[:], bias_cat[:, 0:96], start=False, stop=True)
            coef = sb.tile([128, 96], f32, tag="coef_sb", bufs=4)
            nc.vector.tensor_copy(coef[:], psc[:])

            # sampling grid -> pixel coords + 128, x/y interleaved
            pxs = sb.tile([128, 64], f32, tag="pxs", bufs=4)  # px + 128
            nc.vector.tensor_tensor(
                pxs[:].rearrange("p (s c) -> p s c", c=2),
                coef[:, 0:64].rearrange("p (s c) -> p s c", c=2),
                rp128[:, None, :].to_broadcast([128, 32, 2]), Alu.add)
            nc.vector.tensor_scalar(pxs[:], pxs[:], 127.5, 255.5, Alu.max, Alu.min)
            # pxs >= 127.5 > 0, so the f32->i32 truncation IS the floor
            ri = sb.tile([128, 64], i32, tag="ri", bufs=4)
            nc.vector.tensor_copy(ri[:], pxs[:])
            flr = sb.tile([128, 64], f32, tag="flr", bufs=4)  # floor(px) + 128
            nc.vector.tensor_copy(flr[:], ri[:])
            st = sb.tile([128, 64], f32, tag="st", bufs=4)    # clip start + 128
            nc.vector.tensor_scalar(st[:], flr[:], 128.0, 254.0, Alu.max, Alu.min)

            # gather indices: idx64[q, s, A/B-row] then transpose+interleave
            tbase = sb.tile([128, 32], f32, tag="tbase", bufs=4)
            nc.vector.tensor_scalar(
                tbase[:], st[:].rearrange("p (s c) -> p s c", c=2)[:, :, 1],
                128.0, -16512.0, Alu.mult, Alu.add)
            idx64 = sb.tile([128, 32, 2], f32, tag="idx64", bufs=4)
            nc.vector.tensor_tensor(idx64[:, :, 0], tbase[:],
                                    st[:].rearrange("p (s c) -> p s c", c=2)[:, :, 0], Alu.add)
            nc.vector.tensor_scalar_add(idx64[:, :, 1], idx64[:, :, 0], 128.0)

            # transpose each 16-column idx group with its columns 8x-tiled
            # (stride-0 broadcast), landing the group replicated across all
            # 128 partitions -- the SWDGE wrapped-and-replicated idx layout
            pti_a = ps.tile([128, 2, 128], f32, tag="tp", bufs=2)
            pti_b = ps.tile([128, 2, 128], f32, tag="tp", bufs=2)
            idx64f = idx64[:].rearrange("p s c -> p (s c)")
            for g4 in range(4):
                pti = pti_a if g4 < 2 else pti_b
                nc.tensor.transpose(
                    pti[:, g4 % 2, :],
                    idx64f[:, None, 16 * g4:16 * (g4 + 1)].to_broadcast([128, 8, 16]),
                    ident[:])
            nc.vector.tensor_copy(
                idxt[:, jsl, :, 0:2],
                pti_a[:].rearrange("p g (j q) -> p j q g", q=2))
            nc.vector.tensor_copy(
                idxt[:, jsl, :, 2:4],
                pti_b[:].rearrange("p g (j q) -> p j q g", q=2))



            # ---- pass 2: softmax + bilinear weights ----
            expw = sb.tile([128, 8, 4], f32, tag="expw", bufs=4)
            nc.scalar.activation(expw[:], coef[:, 64:96], Act.Exp)
            den = sb.tile([128, 8], f32, tag="den", bufs=4)
            nc.vector.tensor_reduce(den[:], expw[:], axis=mybir.AxisListType.X, op=Alu.add)
            rden = sb.tile([128, 8], f32, tag="rden", bufs=4)
            nc.vector.reciprocal(rden[:], den[:])
            attn = sb.tile([128, 32], f32, tag="attn", bufs=4)
            nc.vector.tensor_tensor(
                attn[:].rearrange("p (h f) -> p h f", f=4), expw[:],
                rden[:, :, None].to_broadcast([128, 8, 4]), Alu.mult)
            w1 = sb.tile([128, 64], f32, tag="w1", bufs=4)
            nc.vector.tensor_tensor(w1[:], pxs[:], flr[:], Alu.subtract)
            dd = sb.tile([128, 64], f32, tag="dd", bufs=4)
            nc.vector.tensor_tensor(dd[:], flr[:], st[:], Alu.subtract)
            m0 = sb.tile([128, 64], f32, tag="m0", bufs=4)
            nc.vector.tensor_scalar(m0[:], dd[:], 0.0, None, Alu.is_equal)
            mneg = sb.tile([128, 64], f32, tag="mneg", bufs=4)
            nc.vector.tensor_scalar(mneg[:], dd[:], -1.0, None, Alu.is_equal)
            mpos = sb.tile([128, 64], f32, tag="mpos", bufs=4)
            nc.vector.tensor_scalar(mpos[:], dd[:], 1.0, None, Alu.is_equal)
            u0 = sb.tile([128, 64], f32, tag="u0", bufs=4)
            nc.vector.tensor_scalar(u0[:], w1[:], 1.0, -1.0, Alu.subtract, Alu.mult)
            # wA = u0*m0 + u1*mneg ; wB = u1*m0 + u0*mpos   (u1 == w1)
            tA = sb.tile([128, 64], f32, tag="tA", bufs=4)
            nc.vector.tensor_tensor(tA[:], u0[:], m0[:], Alu.mult)
            tB = sb.tile([128, 64], f32, tag="tB", bufs=4)
            nc.vector.tensor_tensor(tB[:], w1[:], mneg[:], Alu.mult)
            wA = sb.tile([128, 32, 2], f32, tag="wA", bufs=4)
            nc.vector.tensor_tensor(wA[:].rearrange("p a b -> p (a b)"), tA[:], tB[:], Alu.add)
            nc.vector.tensor_tensor(tA[:], w1[:], m0[:], Alu.mult)
            nc.vector.tensor_tensor(tB[:], u0[:], mpos[:], Alu.mult)
            wB = sb.tile([128, 32, 2], f32, tag="wB", bufs=4)
            nc.vector.tensor_tensor(wB[:].rearrange("p a b -> p (a b)"), tA[:], tB[:], Alu.add)

            # combine with attention; build wab [128, (AB, h*p, yp)]
            aw = sb.tile([128, 32], f32, tag="aw", bufs=4)
            nc.vector.tensor_tensor(aw[:], attn[:], wA[:, :, 0], Alu.mult)
            bw = sb.tile([128, 32], f32, tag="bw", bufs=4)
            nc.vector.tensor_tensor(bw[:], attn[:], wB[:, :, 0], Alu.mult)
            vcat = sb.tile([128, 32, 2], f32, tag="vcat", bufs=4)
            nc.vector.tensor_copy(vcat[:, :, 0], wA[:, :, 1])
            nc.vector.tensor_copy(vcat[:, :, 1], wB[:, :, 1])
            wab = sb.tile([128, 2, 32, 2], f32, tag="wab", bufs=4)
            nc.vector.tensor_tensor(wab[:, 0], vcat[:],
                                    aw[:, :, None].to_broadcast([128, 32, 2]), Alu.mult)
            nc.vector.tensor_tensor(wab[:, 1], vcat[:],
                                    bw[:, :, None].to_broadcast([128, 32, 2]), Alu.mult)

            # sum of all weights per (q, h) -- border-clip correction for the
            # folded b_v term: sw = sum_{AB,p,yp} wab
            swq = sb.tile([128, 8], f32, tag="swq", bufs=4)
            nc.vector.tensor_reduce(
                swq[:], wab[:].rearrange("p a (h r) c -> p h a r c", h=8),
                axis=mybir.AxisListType.XYZ, op=Alu.add)

            pst2 = ps.tile([128, 256], f32, tag="tp", bufs=2)
            nc.tensor.transpose(pst2[:, 0:128],
                                wab[:].rearrange("p a s c -> p (a s c)"), ident[:])
            pst3 = ps.tile([8, 128], f32, tag="img", bufs=2)
            nc.tensor.transpose(pst3[:], swq[:], ident[:])
            nc.vector.tensor_copy(wabT[:, qsl], pst2[:, 0:128])
            nc.vector.tensor_copy(swT[:, qsl], pst3[:])
            nc.vector.tensor_copy(w_a_i[0:64, jsl], wabT[0:64, qt * 128:qt * 128 + 128:2])
            nc.vector.tensor_copy(w_a_i[64:128, jsl], wabT[0:64, qt * 128 + 1:qt * 128 + 128:2])
            nc.vector.tensor_copy(w_b_i[0:64, jsl], wabT[64:128, qt * 128:qt * 128 + 128:2])
            nc.vector.tensor_copy(w_b_i[64:128, jsl], wabT[64:128, qt * 128 + 1:qt * 128 + 128:2])

            # ---- gather + weighted reduce for this q-tile's 8 chunks;
            # the output GEMM runs per half-tile (64 queries): per-chunk-pair
            # splits lose to the per-kt Ldweights cost on the PE ----
            def out_gemm_half(half):
                hsl = slice(qt * 128 + 64 * half, qt * 128 + 64 * (half + 1))
                pso = ps.tile([64, 256], f32, tag="img", bufs=2)
                for kt, (dh, h) in enumerate((dh, h) for dh in range(2)
                                             for h in range(NH)):
                    nc.tensor.matmul(pso[:], red[:, dh, hsl, h],
                                     wout_bf[:, 2 * h + dh, :],
                                     start=(kt == 0), stop=False)
                nc.tensor.matmul(pso[:], swT[:, hsl], wout_bf[0:8, 16, :],
                                 start=False, stop=False)
                nc.tensor.matmul(pso[:], ones1[:, 0:64], bias_cat[:, 96:352],
                                 start=False, stop=True)
                o_sb = sb.tile([64, 256], f32, tag="o_sb", bufs=2)
                nc.vector.tensor_copy(o_sb[:], pso[:])
                nc.sync.dma_start(out[hsl, :], o_sb[:])

            for g in range(8 * qt, 8 * qt + 8):
                gsl = slice(g * PAIRS_PER_CHUNK, (g + 1) * PAIRS_PER_CHUNK)
                gt_sb = sb.tile([128, PAIRS_PER_CHUNK, 512], bf16, tag="gat", bufs=12)
                if g == NCHUNK - 1:
                    # warm the PE during the last gather so the final reduce
                    # and output GEMM run at full clock; reading the previous
                    # chunk's gathered tile pins these in-phase (dep-free
                    # dummies get hoisted early by the scheduler)
                    dmy2 = ps.tile([128, 256], f32, tag="tp", bufs=2)
                    for r in range(18):
                        nc.tensor.matmul(dmy2[:], prev_gt[:, r % 8, 0:128],
                                         wout_bf[:, r % 16, :],
                                         start=True, stop=True)
                nc.gpsimd.dma_gather(
                    out_ap=gt_sb[:],
                    in_ap=gather_src,
                    idxs_ap=idxt[:].rearrange("p a b c -> p (a b c)")[
                        :, g * (IDX_PER_CHUNK // 16):(g + 1) * (IDX_PER_CHUNK // 16)],
                    num_idxs=IDX_PER_CHUNK,
                    num_idxs_reg=IDX_PER_CHUNK,
                    elem_size=512,
                    elem_step=256,
                )
                if g == 0:
                    # packed Wcomb|bvW load: a Pool-engine WAW pin right
                    # after chunk 0's descriptor-gen drops its 3.2us transfer
                    # exactly into the desc-gen pipeline-fill hole between
                    # the first and second gathers
                    nc.gpsimd.tensor_copy(wout_bf[0:1, 0, 0:1], idxt[0:1, 0:1, 0, 0])
                    nc.scalar.dma_start(wout_bf[:], Wout[:])
                wblkA = sb.tile([128, PAIRS_PER_CHUNK, 16], bf16, tag="wblkA", bufs=4)
                nc.vector.tensor_tensor(
                    wblkA[:], mask16[:, None, :].to_broadcast([128, PAIRS_PER_CHUNK, 16]),
                    w_a_i[:, gsl, None].to_broadcast([128, PAIRS_PER_CHUNK, 16]), Alu.mult)
                wblkB = sb.tile([128, PAIRS_PER_CHUNK, 16], bf16, tag="wblkB", bufs=4)
                nc.vector.tensor_tensor(
                    wblkB[:], mask16[:, None, :].to_broadcast([128, PAIRS_PER_CHUNK, 16]),
                    w_b_i[:, gsl, None].to_broadcast([128, PAIRS_PER_CHUNK, 16]), Alu.mult)

                plo = ps.tile([128, PAIRS_PER_CHUNK * 16], f32, tag="red_lo", bufs=2)
                phi = ps.tile([128, PAIRS_PER_CHUNK * 16], f32, tag="red_hi", bufs=2)
                qsl2 = slice(g * 2 * PAIRS_PER_CHUNK, (g + 1) * 2 * PAIRS_PER_CHUNK)
                for j in range(PAIRS_PER_CHUNK):
                    osl = slice(j * 16, (j + 1) * 16)
                    nc.tensor.matmul(plo[:, osl], gt_sb[:, j, 0:128], wblkA[:, j, :],
                                     start=True, stop=False)
                    nc.tensor.matmul(plo[:, osl], gt_sb[:, j, 256:384], wblkB[:, j, :],
                                     start=False, stop=True)
                nc.scalar.copy(
                    red[:, 0, qsl2, :].rearrange("p a b -> p (a b)"), plo[:])
                for j in range(PAIRS_PER_CHUNK):
                    osl = slice(j * 16, (j + 1) * 16)
                    nc.tensor.matmul(phi[:, osl], gt_sb[:, j, 128:256], wblkA[:, j, :],
                                     start=True, stop=False)
                    nc.tensor.matmul(phi[:, osl], gt_sb[:, j, 384:512], wblkB[:, j, :],
                                     start=False, stop=True)
                nc.scalar.copy(
                    red[:, 1, qsl2, :].rearrange("p a b -> p (a b)"), phi[:])
                if g % 8 == 3:
                    out_gemm_half(0)
                elif g % 8 == 7:
                    out_gemm_half(1)
                prev_gt = gt_sb

    nc.compile()
    return nc


def _get_nc():
    if "nc" not in _CACHE:
        _CACHE["nc"] = _build_bass()
    return _CACHE["nc"]


def _make_in_maps(inputs):
    query = np.ascontiguousarray(np.asarray(inputs["query"], dtype=np.float32))
    refp = np.ascontiguousarray(np.asarray(inputs["reference_points"], dtype=np.float32))
    value = np.ascontiguousarray(
        np.asarray(inputs["value"], dtype=np.float32).astype(ml_dtypes.bfloat16))

    # Wcat [128, 2, 96]: dim-major [p, t, n] view of [W_off*12.8 | W_attn]
    W_off = np.asarray(inputs["W_off"], np.float32) * 12.8
    W_attn = np.asarray(inputs["W_attn"], np.float32)
    wcat = np.concatenate([W_off, W_attn], axis=1)  # [256, 96]
    wcat = wcat.reshape(2, 128, 96).transpose(1, 0, 2)  # [p, t, n]

    # biasp [1, 352]: b_off*12.8 | b_attn | b_out
    biasp = np.concatenate([
        np.asarray(inputs["b_off"], np.float32) * 12.8,
        np.asarray(inputs["b_attn"], np.float32),
        np.asarray(inputs["b_out"], np.float32)])[None, :]

    # Wout [128, 18, 256] bf16: Wcomb planes 0-15 ([p, t, n] of [2048, 256]),
    # bvW plane 16 (rows 0-7), reduce mask plane 17 (cols 0-15)
    W_v = np.asarray(inputs["W_v"], np.float64)
    b_v = np.asarray(inputs["b_v"], np.float64)
    W_out = np.asarray(inputs["W_out"], np.float64).reshape(NH, D, D)
    Wcomb = np.einsum("ij,hjk->hik", W_v, W_out).reshape(NH * D, D)
    bvW = np.einsum("j,hjk->hk", b_v, W_out)
    wout = np.zeros((128, 17, 256), np.float32)
    wout[:, 0:16, :] = Wcomb.reshape(16, 128, 256).transpose(1, 0, 2)
    wout[0:8, 16, :] = bvW

    consts = {
        "Wcat": np.ascontiguousarray(wcat),
        "biasp": np.ascontiguousarray(biasp),
        "Wout": np.ascontiguousarray(wout.astype(ml_dtypes.bfloat16)),
        "maskd": np.ascontiguousarray(_mask16_np()),
    }
    in_maps = []
    for c in range(NCORES):
        b, s = c // 2, c % 2
        qsl = slice(s * QPC, (s + 1) * QPC)
        in_maps.append({
            # host-transposed layout [2, 128, QPC]: qT[t, p, q] = q[q, 128t+p]
            "query": np.ascontiguousarray(
                query[b, qsl].reshape(QPC, 2, 128).transpose(1, 2, 0)),
            "reference_points": np.ascontiguousarray(refp[b, qsl]),
            "value": np.ascontiguousarray(value[b]),
            **consts,
        })
    return in_maps


def _assemble(outs, shape):
    out = np.zeros(shape, dtype=np.float32)
    for c in range(NCORES):
        b, s = c // 2, c % 2
        out[b, s * QPC:(s + 1) * QPC] = outs[c]["out"]
    return out


def kernel(query, reference_points, value, W_off, b_off, W_attn, b_attn,
           W_v, b_v, W_out, b_out, H=128, W=128, **_unused):
    assert int(H) == HW and int(W) == HW
    from concourse.bass_utils import run_bass_kernel_spmd

    inputs = dict(query=query, reference_points=reference_points, value=value,
                  W_off=W_off, b_off=b_off, W_attn=W_attn, b_attn=b_attn,
                  W_v=W_v, b_v=b_v, W_out=W_out, b_out=b_out)
    in_maps = _make_in_maps(inputs)
    nc = _get_nc()
    res = run_bass_kernel_spmd(nc, in_maps, core_ids=list(range(NCORES)))
    outs = res.results if hasattr(res, "results") else res
    B, Q, _ = np.asarray(query).shape
    return _assemble(outs, (B, Q, D))
